# revision 10
# baseline (speedup 1.0000x reference)
"""Trainium2 Bass kernel for nn_MultiHeadAttention_41884521070801.

Sharding: tensor-parallel over heads (4 heads/core) x data-parallel over
batch (B=2) => 8 cores. Each core computes, for its batch element and its
4 heads: QKV projections (+RoPE), causal softmax attention (flash-style,
transposed-scores layout so no transposes are needed on-device), and its
partial output projection (rows of Wo^T). Host sums the 4 partial outputs
per batch element.

All matmuls run in bf16 with fp32 PSUM accumulation. RoPE and softmax
statistics are computed in fp32.
"""

import math

import numpy as np
import ml_dtypes

import concourse.bacc as bacc
import concourse.tile as tile
from concourse import mybir
from concourse.bass_utils import run_bass_kernel_spmd

N_CORES = 8
B = 2
S = 2048
D = 2048
H = 16
HD = 128          # head dim
HLOC = 4          # heads per core
DLOC = HLOC * HD  # 512, per-core slice of the concat-head dim
QCH = 512         # q chunk size
NQC = S // QCH    # 4
NKB = S // 128    # 16 k-blocks
NEB = D // 128    # 16 e-blocks (contraction blocks for projections)
ROPE_THETA = 10000.0
NEG = -1.0e30

F32 = mybir.dt.float32
BF16 = mybir.dt.bfloat16

_BUILD_CACHE = {}

# ablation flags (timing experiments only; correctness requires all True)
FLAGS = {
    "io_dma": True,    # xT chunk loads + output stores
    "exp": True,       # ACT exp (else DVE copy)
    "sums": True,      # row-sum matmuls + normalization
    "mask": True,      # causal ctri adds
    "rope": True,      # rope DVE/ACT work (else direct copy)
    "timing_io": False,  # all data in internal DRAM, tiny external I/O
    "out_gpsimd": False,  # issue output stores on the gpsimd queue
    "act_copies": True,  # psum evacuation copies on ACT (else DVE)
    "attn": True,      # attention phase
    "evac_act": True,  # oc/rb psum evacuation on ACT (else DVE)
    "wo_defer": True,  # interleave prev chunk's Wo into attention
    "wo": True,        # output projection phase
    "proj": True,      # QKV projection phase
}


def _emit_consts(nc, tc, pools, tensors):
    """Emit the one-time constant/weight loads.

    DMA order matters for one-shot latency: the sync queue drains in
    order, so small tables and wq go first (first Q-chain matmuls can
    start after ~4 MB instead of ~16 MB), wo last (needed latest).
    """
    (consts, resid, xc_pool, ps_pool, work, p_pool, rb_pool, oc_pool,
     qcur_pool, ocur_pool, pacc_pool) = pools
    (xT, wqT, wkT, wvT, woT, cosT, sinT, rT, amB, ctri, ident, outp) = tensors
    if True:
        consts.xc0 = [consts.tile([128, QCH], BF16, tag=f"xc0_{e}", name=f"xc0_{e}")
                      for e in range(NEB)]
        if FLAGS["io_dma"]:
            for e in range(NEB):
                nc.sync.dma_start(out=consts.xc0[e], in_=xT[0, e])
        else:
            for e in range(NEB):
                nc.vector.memset(consts.xc0[e], 0.001)
        consts.wq = consts.tile([128, NEB, DLOC], BF16, tag="wq", name="wq")
        consts.wk = consts.tile([128, NEB, DLOC], BF16, tag="wk", name="wk")
        consts.wv = consts.tile([128, NEB, DLOC], BF16, tag="wv", name="wv")
        consts.wo = consts.tile([128, HLOC, D], BF16, tag="wo", name="wo")
        consts.rT = consts.tile([128, HD], BF16, tag="rT", name="rTs")
        nc.sync.dma_start(out=consts.rT, in_=rT[:])
        consts.amB = consts.tile([128, NKB], F32, tag="amB", name="amBs")
        nc.sync.dma_start(out=consts.amB, in_=amB[:])
        consts.ctri = consts.tile([128, 4, QCH], BF16, tag="ctri", name="ctri")
        nc.sync.dma_start(out=consts.ctri, in_=ctri[:].rearrange("p (j q) -> p j q", j=4))
        consts.ident = consts.tile([128, 128], BF16, tag="ident", name="ident")
        nc.sync.dma_start(out=consts.ident, in_=ident[:])
        for e in range(NEB):
            nc.sync.dma_start(out=consts.wq[:, e, :], in_=wqT[e])
        consts.cos = consts.tile([128, S], BF16, tag="cos", name="cos")
        consts.sin = consts.tile([128, S], BF16, tag="sin", name="sin")
        nc.sync.dma_start(out=consts.cos, in_=cosT[:])
        nc.sync.dma_start(out=consts.sin, in_=sinT[:])
        for e in range(NEB):
            nc.sync.dma_start(out=consts.wk[:, e, :], in_=wkT[e])
        for e in range(NEB):
            nc.sync.dma_start(out=consts.wv[:, e, :], in_=wvT[e])
        for hh in range(HLOC):
            nc.sync.dma_start(out=consts.wo[:, hh, :], in_=woT[hh])
        consts.ones_bf = consts.tile([128, 1], BF16, tag="ones_bf", name="ones_bf")
        nc.vector.memset(consts.ones_bf, 1.0)
        consts.ones_row = consts.tile([1, 128], F32, tag="ones_row", name="ones_row")
        nc.vector.memset(consts.ones_row, 1.0)
        # persistent activations (K and V must stay for the whole pass)
        consts.kro = [resid.tile([128, S], BF16, tag=f"kro{h}", name=f"kro{h}")
                      for h in range(HLOC)]
        consts.v = [resid.tile([128, DLOC], BF16, tag=f"v{kb}", name=f"v{kb}")
                    for kb in range(NKB)]


def _emit_body(nc, tc, pools, tensors):
    """Emit one full forward pass (consts already emitted).

    PE executes its instruction stream in order, so cross-engine consumers
    (rope, exp) are software-pipelined: matmuls that depend on another
    engine's output are emitted 1-2 producer-iterations late so the PE
    always has independent work queued.
    """
    (consts, resid, xc_pool, ps_pool, work, p_pool, rb_pool, oc_pool,
     qcur_pool, ocur_pool, pacc_pool) = pools
    (xT, wqT, wkT, wvT, woT, cosT, sinT, rT, amB, ctri, ident, outp) = tensors

    wq, wk, wv, wo = consts.wq, consts.wk, consts.wv, consts.wo
    cos_s, sin_s, rT_s, amB_s, ctri_s = (
        consts.cos, consts.sin, consts.rT, consts.amB, consts.ctri)
    kro, v_s = consts.kro, consts.v

    def rope_pre(src_ps, qc):
        """ACT-copy psum -> bf16 sbuf (stage 1 of rope)."""
        qf = work.tile([128, QCH], BF16, tag="ropef", name="ropef", bufs=4)
        if FLAGS["act_copies"]:
            nc.scalar.copy(qf, src_ps)
        else:
            nc.vector.tensor_copy(qf, src_ps)
        return qf

    def rope_rot(qf):
        """PE rotate-half matmul (stage 2)."""
        rot = ps_pool.tile([128, QCH], F32, tag="ps", name="ps")
        nc.tensor.matmul(rot, lhsT=rT_s, rhs=qf, start=True, stop=True)
        return rot

    def rope_fin(qf, rot, dst_ap, qc):
        """DVE combine (stage 3)."""
        t1 = work.tile([128, QCH], BF16, tag="ropet1", name="ropet1", bufs=3)
        nc.vector.tensor_mul(t1, qf, cos_s[:, qc * QCH:(qc + 1) * QCH])
        t2 = work.tile([128, QCH], BF16, tag="ropet2", name="ropet2", bufs=3)
        nc.vector.tensor_mul(t2, rot, sin_s[:, qc * QCH:(qc + 1) * QCH])
        nc.vector.tensor_add(dst_ap, t1, t2)

    # cross-chunk deferred work: the previous chunk's Wo units are emitted
    # interleaved into this chunk's attention (which is ACT-gated per
    # k-block), and the last head's sums matmul + normalize are deferred
    # into the next chunk's projection phase so the chunk-end serial chain
    # (exp -> add -> sums -> recip -> bcast -> mul) overlaps dense PE work.
    norm_q = []        # (ops, sps, ot) awaiting recip/bcast/mul
    wo_q = []          # deferred Wo unit closures from the previous chunk

    def emit_normalize():
        ops0, sps0, ot0 = norm_q.pop(0)
        if FLAGS["sums"]:
            # normalize: o = ops * (1/sums), broadcast along partitions
            # via a K=1 outer-product matmul
            r_row = rb_pool.tile([1, QCH], F32, tag="rrow", name="rrow")
            nc.vector.reciprocal(r_row, sps0)
            rb_ps = ps_pool.tile([128, QCH], F32, tag="ps", name="ps")
            nc.tensor.matmul(rb_ps, lhsT=consts.ones_row, rhs=r_row,
                             start=True, stop=True)
            rb_sb = rb_pool.tile([128, QCH], F32, tag="rb", name="rb")
            if FLAGS["evac_act"]:
                nc.scalar.copy(rb_sb, rb_ps)
            else:
                nc.vector.tensor_copy(rb_sb, rb_ps)
            nc.vector.tensor_mul(ot0[:], ops0, rb_sb)
        else:
            nc.vector.tensor_copy(ot0[:], ops0)

    for qc in range(NQC):
        # ---- load x^T chunk: 16 tiles [128 e, 512 q] ----
        # qc==0 tiles are persistent and were loaded up front in
        # _emit_consts (x is invariant across repeat iterations)
        if qc == 0:
            xc = consts.xc0
        else:
            xc = []
            for e in range(NEB):
                t = xc_pool.tile([128, QCH], BF16, tag="xc", name="xc")
                if FLAGS["io_dma"]:
                    nc.sync.dma_start(out=t, in_=xT[qc, e])
                else:
                    nc.vector.memset(t, 0.001)
                xc.append(t)

        # ---- QKV projections, rope software-pipelined behind them ----
        qcur = []
        if FLAGS["proj"]:
            # chains: (weight, head, dst_ap) for Q then K
            chains = []
            for h in range(HLOC):
                qt = qcur_pool.tile([128, QCH], BF16, tag="qcur", name="qcur")
                qcur.append(qt)
                chains.append((wq, h, qt[:]))
            for h in range(HLOC):
                chains.append((wk, h, kro[h][:, qc * QCH:(qc + 1) * QCH]))

            pending = []  # (qf, pp, dst_ap) awaiting rot+fin
            def drain_pending():
                qf, dst_ap = pending.pop(0)
                rot = rope_rot(qf)
                rope_fin(qf, rot, dst_ap, qc)

            for ci, (w_s, h, dst_ap) in enumerate(chains):
                pp = ps_pool.tile([128, QCH], F32, tag="ps", name="ps")
                for e in range(NEB):
                    nc.tensor.matmul(
                        pp, lhsT=w_s[:, e, h * HD:(h + 1) * HD], rhs=xc[e],
                        start=(e == 0), stop=(e == NEB - 1))
                if ci == 0 and norm_q:
                    # previous chunk's last-head normalize: recip ran on DVE
                    # while the first Q chain streamed, so the bcast MM here
                    # does not stall
                    emit_normalize()
                qf = rope_pre(pp, qc)
                pending.append((qf, dst_ap))
                if len(pending) >= 2:
                    drain_pending()

            # ---- V (natural [k, d] layout) ----
            for kb4 in range(4):
                kb = qc * 4 + kb4
                pp = ps_pool.tile([128, DLOC], F32, tag="ps", name="ps")
                for e in range(NEB):
                    nc.tensor.matmul(
                        pp, lhsT=xc[e][:, kb4 * 128:(kb4 + 1) * 128],
                        rhs=wv[:, e, :],
                        start=(e == 0), stop=(e == NEB - 1))
                if FLAGS["act_copies"]:
                    nc.scalar.copy(v_s[kb], pp)
                else:
                    nc.vector.tensor_copy(v_s[kb], pp)
                while pending:
                    drain_pending()
            while pending:
                drain_pending()
        else:
            for h in range(HLOC):
                qt = qcur_pool.tile([128, QCH], BF16, tag="qcur", name="qcur")
                qcur.append(qt)
                nc.vector.memset(qt, 0.01)
            while norm_q:
                emit_normalize()

        # ---- attention: scores+exp pipelined 2 ahead of PV ----
        nkb = 4 * qc + 4
        ocur = []
        # interleave the previous chunk's Wo units over this chunk's
        # ACT-gated attention iterations
        n_iters = HLOC * nkb
        wo_stride = max(1, n_iters // max(1, len(wo_q))) if wo_q else 0
        iter_no = 0

        for h in range(HLOC if FLAGS["attn"] else 0):
            ops = ps_pool.tile([128, QCH], F32, tag="ps", name="ps")

            def emit_scores(kb):
                off = max(0, (kb - 4 * qc) * 128)
                diag = kb >= 4 * qc
                s_ps = ps_pool.tile([128, QCH], F32, tag="ps", name="ps")
                nc.tensor.matmul(
                    s_ps[:, off:], lhsT=kro[h][:, kb * 128:(kb + 1) * 128],
                    rhs=qcur[h][:, off:], start=True,
                    stop=not (FLAGS["mask"] and diag))
                if FLAGS["mask"] and diag:
                    # accumulate the additive causal triangle: I.T @ tri.
                    # tri is zero beyond the diagonal 128-block, so only
                    # the [off:off+128] sub-range needs the add.
                    j = kb - 4 * qc
                    nc.tensor.matmul(
                        s_ps[:, off:off + 128], lhsT=consts.ident,
                        rhs=ctri_s[:, j, off:off + 128],
                        start=False, stop=True)
                p_sb = p_pool.tile([128, QCH], BF16, tag="p", name="p")
                if FLAGS["exp"]:
                    nc.scalar.activation(
                        p_sb[:, off:], s_ps[:, off:],
                        mybir.ActivationFunctionType.Exp,
                        bias=amB_s[:, kb:kb + 1], scale=1.0)
                else:
                    nc.vector.tensor_copy(p_sb[:, off:], s_ps[:, off:])
                return (p_sb, off)

            sps = ps_pool.tile([1, QCH], F32, tag="ps", name="ps")

            LOOKAHEAD = 2
            fifo = [emit_scores(kb) for kb in range(min(LOOKAHEAD, nkb))]
            for kb in range(nkb):
                if kb + LOOKAHEAD < nkb:
                    fifo.append(emit_scores(kb + LOOKAHEAD))
                p_sb, off = fifo.pop(0)
                nc.tensor.matmul(
                    ops[:, off:], lhsT=v_s[kb][:, h * HD:(h + 1) * HD],
                    rhs=p_sb[:, off:],
                    start=(kb == 0), stop=(kb == nkb - 1), skip_group_check=True)
                if FLAGS["sums"]:
                    nc.tensor.matmul(
                        sps[:, off:], lhsT=consts.ones_bf, rhs=p_sb[:, off:],
                        start=(kb == 0), stop=(kb == nkb - 1),
                        skip_group_check=True)
                if kb == 1 and norm_q:
                    # drain the previous head's normalize: its inputs are
                    # ready, so the PE bcast matmul does not stall, and the
                    # held psum banks free up early
                    emit_normalize()
                iter_no += 1
                if wo_q and wo_stride and iter_no % wo_stride == 0:
                    wo_q.pop(0)()

            ot = ocur_pool.tile([128, QCH], BF16, tag="ocur", name="ocur")
            ocur.append(ot)
            norm_q.append((ops, sps, ot))

        # any Wo units not consumed by the interleave
        while wo_q:
            wo_q.pop(0)()

        # ---- build this chunk's deferred Wo units ----
        if not (FLAGS["wo"] and FLAGS["attn"]):
            while norm_q:
                emit_normalize()
            continue

        def make_wo_unit(qc0, ocur0, qb4, ec):
            def emit():
                qb = qc0 * 4 + qb4
                op_ps = ps_pool.tile([128, QCH], F32, tag="ps", name="ps")
                for h in range(HLOC):
                    nc.tensor.matmul(
                        op_ps,
                        lhsT=ocur0[h][:, qb4 * 128:(qb4 + 1) * 128],
                        rhs=wo[:, h, ec * QCH:(ec + 1) * QCH],
                        start=(h == 0), stop=(h == HLOC - 1))
                oc = oc_pool.tile([128, QCH], BF16, tag="oc", name="oc")
                if FLAGS["evac_act"]:
                    nc.scalar.copy(oc, op_ps)
                else:
                    nc.vector.tensor_copy(oc, op_ps)
                if FLAGS["io_dma"]:
                    eng = nc.gpsimd if FLAGS["out_gpsimd"] else nc.sync
                    eng.dma_start(out=outp[qb, ec], in_=oc)
            return emit

        for qb4 in range(QCH // 128):
            for ec in range(D // QCH):
                wo_q.append(make_wo_unit(qc, ocur, qb4, ec))
        if not FLAGS["wo_defer"]:
            while norm_q:
                emit_normalize()
            while wo_q:
                wo_q.pop(0)()

    # ---- drain the tail: last chunk's normalize + Wo ----
    while norm_q:
        emit_normalize()
    while wo_q:
        wo_q.pop(0)()


def build_nc(repeat=1):
    key = (repeat, tuple(sorted(FLAGS.items())))
    if key in _BUILD_CACHE:
        return _BUILD_CACHE[key]
    nc = bacc.Bacc("TRN2", target_bir_lowering=False, debug=False,
                   num_devices=N_CORES)
    if FLAGS["timing_io"]:
        kind = "Internal"
        dummy_in = nc.dram_tensor("dummy_in", [1, 4], F32, kind="ExternalInput")
        dummy_out = nc.dram_tensor("dummy_out", [1, 4], F32, kind="ExternalOutput")
    else:
        kind = "ExternalInput"
    xT = nc.dram_tensor("xT", [NQC, NEB, 128, QCH], BF16, kind=kind)
    wqT = nc.dram_tensor("wqT", [NEB, 128, DLOC], BF16, kind=kind)
    wkT = nc.dram_tensor("wkT", [NEB, 128, DLOC], BF16, kind=kind)
    wvT = nc.dram_tensor("wvT", [NEB, 128, DLOC], BF16, kind=kind)
    woT = nc.dram_tensor("woT", [HLOC, 128, D], BF16, kind=kind)
    cosT = nc.dram_tensor("cosT", [HD, S], BF16, kind=kind)
    sinT = nc.dram_tensor("sinT", [HD, S], BF16, kind=kind)
    rT = nc.dram_tensor("rT", [HD, HD], BF16, kind=kind)
    amB = nc.dram_tensor("amB", [128, NKB], F32, kind=kind)
    ctri = nc.dram_tensor("tri", [128, 4 * QCH], BF16, kind=kind)
    ident = nc.dram_tensor("ident", [128, 128], BF16, kind=kind)
    if FLAGS["timing_io"]:
        outp = nc.dram_tensor("outp", [S // 128, D // QCH, 128, QCH], BF16,
                              kind="Internal")
    else:
        outp = nc.dram_tensor("outp", [S // 128, D // QCH, 128, QCH], BF16,
                              kind="ExternalOutput")
    tensors = (xT, wqT, wkT, wvT, woT, cosT, sinT, rT, amB, ctri, ident, outp)

    from contextlib import ExitStack
    with tile.TileContext(nc) as tc, ExitStack() as ctx:
        consts = ctx.enter_context(tc.tile_pool(name="consts", bufs=1))
        resid = ctx.enter_context(tc.tile_pool(name="resid", bufs=1))
        xc_pool = ctx.enter_context(tc.tile_pool(name="xc", bufs=20))
        ps_pool = ctx.enter_context(tc.tile_pool(name="ps", bufs=8, space="PSUM"))
        work = ctx.enter_context(tc.tile_pool(name="work", bufs=2))
        p_pool = ctx.enter_context(tc.tile_pool(name="p", bufs=6))
        rb_pool = ctx.enter_context(tc.tile_pool(name="rb", bufs=2))
        oc_pool = ctx.enter_context(tc.tile_pool(name="oc", bufs=3))
        qcur_pool = ctx.enter_context(tc.tile_pool(name="qcur", bufs=8))
        ocur_pool = ctx.enter_context(tc.tile_pool(name="ocur", bufs=8))
        pacc_pool = ctx.enter_context(tc.tile_pool(name="pacc", bufs=3))
        pools = (consts, resid, xc_pool, ps_pool, work, p_pool, rb_pool,
                 oc_pool, qcur_pool, ocur_pool, pacc_pool)
        _emit_consts(nc, tc, pools, tensors)
        if FLAGS["timing_io"]:
            dsb = pools[4].tile([1, 4], F32, tag="dummy", name="dummy")
            nc.sync.dma_start(out=dsb, in_=dummy_in[:])
            nc.sync.dma_start(out=dummy_out[:], in_=dsb)
        if repeat == 1:
            _emit_body(nc, tc, pools, tensors)
        else:
            with tc.For_i(0, repeat, 1, hint_engines=(mybir.EngineType.PE, mybir.EngineType.DVE, mybir.EngineType.Activation)):
                _emit_body(nc, tc, pools, tensors)
    nc.compile()
    _BUILD_CACHE[key] = nc
    return nc


def make_core_inputs(hidden_states, attention_mask, Wq, Wk, Wv, Wo):
    """Host-side prep: returns list of 8 in_maps."""
    f32 = np.float32
    bf16 = ml_dtypes.bfloat16
    hidden_states = np.asarray(hidden_states, dtype=f32)
    attention_mask = np.asarray(attention_mask, dtype=f32)
    Wq = np.asarray(Wq, dtype=f32)
    Wk = np.asarray(Wk, dtype=f32)
    Wv = np.asarray(Wv, dtype=f32)
    Wo = np.asarray(Wo, dtype=f32)

    # rope tables, [hd, S] layout
    invf = 1.0 / (ROPE_THETA ** (np.arange(0, HD, 2, dtype=f32) / HD))
    t = np.arange(S, dtype=f32)
    fr = t[:, None] * invf[None, :]            # [S, hd/2]
    emb = np.concatenate([fr, fr], axis=-1)    # [S, hd]
    cosT = np.cos(emb).T.astype(bf16).copy()   # [hd, S]
    sinT = np.sin(emb).T.astype(bf16).copy()

    # rotate-half matrix: (R @ x)[i] = -x[i+64] (i<64), x[i-64] (i>=64)
    R = np.zeros((HD, HD), dtype=f32)
    half = HD // 2
    for i in range(half):
        R[i, i + half] = -1.0
        R[i + half, i] = 1.0
    rT = R.T.copy()

    # causal additive triangle for the diagonal 128x128 sub-block
    p = np.arange(128)[:, None]
    c = np.arange(QCH)[None, :]
    tri = np.zeros((128, 4, QCH), dtype=np.float32)
    for j in range(4):
        qrel = c - 128 * j
        tri[:, j, :] = np.where((qrel >= 0) & (qrel < 128) & (p > qrel), NEG, 0.0)
    tri = tri.reshape(128, 4 * QCH).astype(bf16)
    ident = np.eye(128, dtype=np.float32).astype(bf16)

    scale = 1.0 / math.sqrt(HD)
    in_maps = []
    for core in range(N_CORES):
        b = core // (N_CORES // B)
        hg = core % (N_CORES // B)
        rows = slice(hg * DLOC, (hg + 1) * DLOC)
        amv = np.where(attention_mask[b] == 0, NEG, attention_mask[b]).astype(f32)
        in_maps.append({
            "xT": np.ascontiguousarray(
                hidden_states[b].T.reshape(NEB, 128, NQC, QCH)
                .transpose(2, 0, 1, 3)).astype(bf16),
            "wqT": (Wq[rows, :] * scale).T.reshape(NEB, 128, DLOC).astype(bf16),
            "wkT": Wk[rows, :].T.reshape(NEB, 128, DLOC).astype(bf16),
            "wvT": Wv[rows, :].T.reshape(NEB, 128, DLOC).astype(bf16),
            "woT": Wo[:, rows].T.reshape(HLOC, 128, D).astype(bf16),
            "cosT": cosT,
            "sinT": sinT,
            "rT": rT.astype(bf16),
            "amB": amv.reshape(NKB, 128).T.copy(),
            "tri": tri,
            "ident": ident,
        })
    return in_maps


def kernel(**inputs):
    nc = build_nc()
    in_maps = make_core_inputs(**inputs)
    res = run_bass_kernel_spmd(nc, in_maps, list(range(N_CORES)))
    out = np.zeros((B, S, D), dtype=np.float32)
    ncb = N_CORES // B
    for core in range(N_CORES):
        r = res.results[core]["outp"]          # [16, 4, 128, 512] tiled, bf16
        out[core // ncb] += r.astype(np.float32).transpose(0, 2, 1, 3).reshape(S, D)
    return out



# revision 11
# speedup vs baseline: 1.0313x; 1.0313x over previous
"""Trainium2 Bass kernel for nn_MultiHeadAttention_41884521070801.

Sharding: tensor-parallel over heads (4 heads/core) x data-parallel over
batch (B=2) => 8 cores. Each core computes, for its batch element and its
4 heads: QKV projections (+RoPE), causal softmax attention (flash-style,
transposed-scores layout so no transposes are needed on-device), and its
partial output projection (rows of Wo^T). Host sums the 4 partial outputs
per batch element.

All matmuls run in bf16 with fp32 PSUM accumulation. RoPE and softmax
statistics are computed in fp32.
"""

import math

import numpy as np
import ml_dtypes

import concourse.bacc as bacc
import concourse.tile as tile
from concourse import mybir
from concourse.bass_utils import run_bass_kernel_spmd

N_CORES = 8
B = 2
S = 2048
D = 2048
H = 16
HD = 128          # head dim
HLOC = 4          # heads per core
DLOC = HLOC * HD  # 512, per-core slice of the concat-head dim
QCH = 512         # q chunk size
NQC = S // QCH    # 4
NKB = S // 128    # 16 k-blocks
NEB = D // 128    # 16 e-blocks (contraction blocks for projections)
ROPE_THETA = 10000.0
NEG = -1.0e30

F32 = mybir.dt.float32
BF16 = mybir.dt.bfloat16

_BUILD_CACHE = {}

# ablation flags (timing experiments only; correctness requires all True)
FLAGS = {
    "io_dma": True,    # xT chunk loads + output stores
    "exp": True,       # ACT exp (else DVE copy)
    "sums": True,      # row-sum matmuls + normalization
    "mask": True,      # causal ctri adds
    "rope": True,      # rope DVE/ACT work (else direct copy)
    "timing_io": False,  # all data in internal DRAM, tiny external I/O
    "out_gpsimd": False,  # issue output stores on the gpsimd queue
    "act_copies": True,  # psum evacuation copies on ACT (else DVE)
    "attn": True,      # attention phase
    "evac_act": True,  # oc/rb psum evacuation on ACT (else DVE)
    "wo_defer": True,  # interleave prev chunk's Wo into attention
    "wo": True,        # output projection phase
    "proj": True,      # QKV projection phase
}


def _emit_consts(nc, tc, pools, tensors):
    """Emit the one-time constant/weight loads.

    DMA order matters for one-shot latency: the sync queue drains in
    order, so small tables and wq go first (first Q-chain matmuls can
    start after ~4 MB instead of ~16 MB), wo last (needed latest).
    """
    (consts, resid, xc_pool, ps_pool, work, p_pool, rb_pool, oc_pool,
     qcur_pool, ocur_pool, pacc_pool) = pools
    (xT, wqT, wkT, wvT, woT, cosT, sinT, rT, amB, eamF, eamH, ctri, ident,
     outp) = tensors
    if True:
        consts.xc0 = [consts.tile([128, QCH], BF16, tag=f"xc0_{e}", name=f"xc0_{e}")
                      for e in range(NEB)]
        if FLAGS["io_dma"]:
            for e in range(NEB):
                nc.sync.dma_start(out=consts.xc0[e], in_=xT[0, e])
        else:
            for e in range(NEB):
                nc.vector.memset(consts.xc0[e], 0.001)
        consts.wq = consts.tile([128, NEB, DLOC], BF16, tag="wq", name="wq")
        consts.wk = consts.tile([128, NEB, DLOC], BF16, tag="wk", name="wk")
        consts.wv = consts.tile([128, NEB, DLOC], BF16, tag="wv", name="wv")
        consts.wo = consts.tile([128, HLOC, D], BF16, tag="wo", name="wo")
        consts.rT = consts.tile([128, HD], BF16, tag="rT", name="rTs")
        nc.sync.dma_start(out=consts.rT, in_=rT[:])
        consts.amB = consts.tile([128, NKB], F32, tag="amB", name="amBs")
        nc.sync.dma_start(out=consts.amB, in_=amB[:])
        consts.eamF = consts.tile([128, NKB], F32, tag="eamF", name="eamFs")
        nc.sync.dma_start(out=consts.eamF, in_=eamF[:])
        consts.eamH = consts.tile([128, NKB], BF16, tag="eamH", name="eamHs")
        nc.sync.dma_start(out=consts.eamH, in_=eamH[:])
        consts.ctri = consts.tile([128, 4, QCH], BF16, tag="ctri", name="ctri")
        nc.sync.dma_start(out=consts.ctri, in_=ctri[:].rearrange("p (j q) -> p j q", j=4))
        consts.ident = consts.tile([128, 128], BF16, tag="ident", name="ident")
        nc.sync.dma_start(out=consts.ident, in_=ident[:])
        for e in range(NEB):
            nc.sync.dma_start(out=consts.wq[:, e, :], in_=wqT[e])
        consts.cos = consts.tile([128, S], BF16, tag="cos", name="cos")
        consts.sin = consts.tile([128, S], BF16, tag="sin", name="sin")
        nc.sync.dma_start(out=consts.cos, in_=cosT[:])
        nc.sync.dma_start(out=consts.sin, in_=sinT[:])
        for e in range(NEB):
            nc.sync.dma_start(out=consts.wk[:, e, :], in_=wkT[e])
        for e in range(NEB):
            nc.sync.dma_start(out=consts.wv[:, e, :], in_=wvT[e])
        for hh in range(HLOC):
            nc.sync.dma_start(out=consts.wo[:, hh, :], in_=woT[hh])
        consts.ones_bf = consts.tile([128, 1], BF16, tag="ones_bf", name="ones_bf")
        nc.vector.memset(consts.ones_bf, 1.0)
        consts.ones_row = consts.tile([1, 128], F32, tag="ones_row", name="ones_row")
        nc.vector.memset(consts.ones_row, 1.0)
        # persistent activations (K and V must stay for the whole pass)
        consts.kro = [resid.tile([128, S], BF16, tag=f"kro{h}", name=f"kro{h}")
                      for h in range(HLOC)]
        consts.v = [resid.tile([128, DLOC], BF16, tag=f"v{kb}", name=f"v{kb}")
                    for kb in range(NKB)]


def _emit_body(nc, tc, pools, tensors):
    """Emit one full forward pass (consts already emitted).

    PE executes its instruction stream in order, so cross-engine consumers
    (rope, exp) are software-pipelined: matmuls that depend on another
    engine's output are emitted 1-2 producer-iterations late so the PE
    always has independent work queued.
    """
    (consts, resid, xc_pool, ps_pool, work, p_pool, rb_pool, oc_pool,
     qcur_pool, ocur_pool, pacc_pool) = pools
    (xT, wqT, wkT, wvT, woT, cosT, sinT, rT, amB, eamF, eamH, ctri, ident,
     outp) = tensors

    wq, wk, wv, wo = consts.wq, consts.wk, consts.wv, consts.wo
    cos_s, sin_s, rT_s, amB_s, ctri_s = (
        consts.cos, consts.sin, consts.rT, consts.amB, consts.ctri)
    kro, v_s = consts.kro, consts.v

    def rope_pre(src_ps, qc):
        """ACT-copy psum -> bf16 sbuf (stage 1 of rope)."""
        qf = work.tile([128, QCH], BF16, tag="ropef", name="ropef", bufs=4)
        if FLAGS["act_copies"]:
            nc.scalar.copy(qf, src_ps)
        else:
            nc.vector.tensor_copy(qf, src_ps)
        return qf

    def rope_rot(qf):
        """PE rotate-half matmul (stage 2)."""
        rot = ps_pool.tile([128, QCH], F32, tag="ps", name="ps")
        nc.tensor.matmul(rot, lhsT=rT_s, rhs=qf, start=True, stop=True)
        return rot

    def rope_fin(qf, rot, dst_ap, qc):
        """DVE combine (stage 3)."""
        t1 = work.tile([128, QCH], BF16, tag="ropet1", name="ropet1", bufs=3)
        nc.vector.tensor_mul(t1, qf, cos_s[:, qc * QCH:(qc + 1) * QCH])
        t2 = work.tile([128, QCH], BF16, tag="ropet2", name="ropet2", bufs=3)
        nc.vector.tensor_mul(t2, rot, sin_s[:, qc * QCH:(qc + 1) * QCH])
        nc.vector.tensor_add(dst_ap, t1, t2)

    # cross-chunk deferred work: the previous chunk's Wo units are emitted
    # interleaved into this chunk's attention (which is ACT-gated per
    # k-block), and the last head's sums matmul + normalize are deferred
    # into the next chunk's projection phase so the chunk-end serial chain
    # (exp -> add -> sums -> recip -> bcast -> mul) overlaps dense PE work.
    norm_q = []        # (ops, sps, ot) awaiting recip/bcast/mul
    wo_q = []          # deferred Wo unit closures from the previous chunk

    def emit_normalize():
        ops0, sps0, ot0 = norm_q.pop(0)
        if FLAGS["sums"]:
            # normalize: o = ops * (1/sums), broadcast along partitions
            # via a K=1 outer-product matmul
            r_row = rb_pool.tile([1, QCH], F32, tag="rrow", name="rrow")
            nc.vector.reciprocal(r_row, sps0)
            rb_ps = ps_pool.tile([128, QCH], F32, tag="ps", name="ps")
            nc.tensor.matmul(rb_ps, lhsT=consts.ones_row, rhs=r_row,
                             start=True, stop=True)
            rb_sb = rb_pool.tile([128, QCH], F32, tag="rb", name="rb")
            if FLAGS["evac_act"]:
                nc.scalar.copy(rb_sb, rb_ps)
            else:
                nc.vector.tensor_copy(rb_sb, rb_ps)
            nc.vector.tensor_mul(ot0[:], ops0, rb_sb)
        else:
            nc.vector.tensor_copy(ot0[:], ops0)

    for qc in range(NQC):
        # ---- load x^T chunk: 16 tiles [128 e, 512 q] ----
        # qc==0 tiles are persistent and were loaded up front in
        # _emit_consts (x is invariant across repeat iterations)
        if qc == 0:
            xc = consts.xc0
        else:
            xc = []
            for e in range(NEB):
                t = xc_pool.tile([128, QCH], BF16, tag="xc", name="xc")
                if FLAGS["io_dma"]:
                    nc.sync.dma_start(out=t, in_=xT[qc, e])
                else:
                    nc.vector.memset(t, 0.001)
                xc.append(t)

        # ---- QKV projections, rope software-pipelined behind them ----
        qcur = []
        if FLAGS["proj"]:
            # chains: (weight, head, dst_ap) for Q then K
            chains = []
            for h in range(HLOC):
                qt = qcur_pool.tile([128, QCH], BF16, tag="qcur", name="qcur")
                qcur.append(qt)
                chains.append((wq, h, qt[:]))
            for h in range(HLOC):
                chains.append((wk, h, kro[h][:, qc * QCH:(qc + 1) * QCH]))

            pending = []  # (qf, pp, dst_ap) awaiting rot+fin
            def drain_pending():
                qf, dst_ap = pending.pop(0)
                rot = rope_rot(qf)
                rope_fin(qf, rot, dst_ap, qc)

            for ci, (w_s, h, dst_ap) in enumerate(chains):
                pp = ps_pool.tile([128, QCH], F32, tag="ps", name="ps")
                for e in range(NEB):
                    nc.tensor.matmul(
                        pp, lhsT=w_s[:, e, h * HD:(h + 1) * HD], rhs=xc[e],
                        start=(e == 0), stop=(e == NEB - 1))
                if ci == 0 and norm_q:
                    # previous chunk's last-head normalize: recip ran on DVE
                    # while the first Q chain streamed, so the bcast MM here
                    # does not stall
                    emit_normalize()
                qf = rope_pre(pp, qc)
                pending.append((qf, dst_ap))
                if len(pending) >= 2:
                    drain_pending()

            # ---- V (natural [k, d] layout) ----
            for kb4 in range(4):
                kb = qc * 4 + kb4
                pp = ps_pool.tile([128, DLOC], F32, tag="ps", name="ps")
                for e in range(NEB):
                    nc.tensor.matmul(
                        pp, lhsT=xc[e][:, kb4 * 128:(kb4 + 1) * 128],
                        rhs=wv[:, e, :],
                        start=(e == 0), stop=(e == NEB - 1))
                if FLAGS["act_copies"]:
                    # fold the per-key softmax bias factor e^am into V
                    nc.scalar.activation(
                        v_s[kb], pp, mybir.ActivationFunctionType.Copy,
                        scale=consts.eamF[:, kb:kb + 1])
                else:
                    nc.vector.tensor_copy(v_s[kb], pp)
                while pending:
                    drain_pending()
            while pending:
                drain_pending()
        else:
            for h in range(HLOC):
                qt = qcur_pool.tile([128, QCH], BF16, tag="qcur", name="qcur")
                qcur.append(qt)
                nc.vector.memset(qt, 0.01)
            while norm_q:
                emit_normalize()

        # ---- attention: scores+exp pipelined 2 ahead of PV ----
        nkb = 4 * qc + 4
        ocur = []
        # interleave the previous chunk's Wo units over this chunk's
        # ACT-gated attention iterations
        n_iters = HLOC * nkb
        wo_stride = max(1, n_iters // max(1, len(wo_q))) if wo_q else 0
        iter_no = 0

        for h in range(HLOC if FLAGS["attn"] else 0):
            ops = ps_pool.tile([128, QCH], F32, tag="ps", name="ps")

            def emit_scores_pair(kp):
                # two k-blocks share one 2-bank psum tile so a single exp
                # instruction covers both (the additive mask bias is folded
                # into V and the sums weights as e^am, so no per-block bias)
                s2 = ps_pool.tile([128, 2, QCH], F32, tag="ps2", name="ps2",
                                  bufs=2)
                offs = []
                for b in range(2):
                    kb = 2 * kp + b
                    off = max(0, (kb - 4 * qc) * 128)
                    diag = kb >= 4 * qc
                    nc.tensor.matmul(
                        s2[:, b, off:], lhsT=kro[h][:, kb * 128:(kb + 1) * 128],
                        rhs=qcur[h][:, off:], start=True,
                        stop=not (FLAGS["mask"] and diag))
                    if FLAGS["mask"] and diag:
                        # the additive causal triangle: I.T @ tri; tri is
                        # zero beyond the diagonal 128-block
                        j = kb - 4 * qc
                        nc.tensor.matmul(
                            s2[:, b, off:off + 128], lhsT=consts.ident,
                            rhs=ctri_s[:, j, off:off + 128],
                            start=False, stop=True)
                    offs.append(off)
                p2 = p_pool.tile([128, 2, QCH], BF16, tag="p", name="p",
                                 bufs=4)
                off0 = offs[0]
                if FLAGS["exp"]:
                    nc.scalar.activation(
                        p2[:, :, off0:], s2[:, :, off0:],
                        mybir.ActivationFunctionType.Exp)
                else:
                    nc.vector.tensor_copy(p2[:, :, off0:], s2[:, :, off0:])
                return (p2, offs)

            sps = ps_pool.tile([1, QCH], F32, tag="ps", name="ps")

            nkp = nkb // 2
            fifo = [emit_scores_pair(kp) for kp in range(min(1, nkp))]
            for kp in range(nkp):
                if kp + 1 < nkp:
                    fifo.append(emit_scores_pair(kp + 1))
                p2, offs = fifo.pop(0)
                for b in range(2):
                    kb = 2 * kp + b
                    off = offs[b]
                    nc.tensor.matmul(
                        ops[:, off:], lhsT=v_s[kb][:, h * HD:(h + 1) * HD],
                        rhs=p2[:, b, off:],
                        start=(kb == 0), stop=(kb == nkb - 1),
                        skip_group_check=True)
                    if FLAGS["sums"]:
                        nc.tensor.matmul(
                            sps[:, off:], lhsT=consts.eamH[:, kb:kb + 1],
                            rhs=p2[:, b, off:],
                            start=(kb == 0), stop=(kb == nkb - 1),
                            skip_group_check=True)
                    iter_no += 1
                    if wo_q and wo_stride and iter_no % wo_stride == 0:
                        wo_q.pop(0)()
                if kp == 0 and norm_q:
                    # drain the previous head's normalize: its inputs are
                    # ready, so the PE bcast matmul does not stall, and the
                    # held psum banks free up early
                    emit_normalize()

            ot = ocur_pool.tile([128, QCH], BF16, tag="ocur", name="ocur")
            ocur.append(ot)
            norm_q.append((ops, sps, ot))

        # any Wo units not consumed by the interleave
        while wo_q:
            wo_q.pop(0)()

        # ---- build this chunk's deferred Wo units ----
        if not (FLAGS["wo"] and FLAGS["attn"]):
            while norm_q:
                emit_normalize()
            continue

        def make_wo_unit(qc0, ocur0, qb4, ec):
            def emit():
                qb = qc0 * 4 + qb4
                op_ps = ps_pool.tile([128, QCH], F32, tag="ps", name="ps")
                for h in range(HLOC):
                    nc.tensor.matmul(
                        op_ps,
                        lhsT=ocur0[h][:, qb4 * 128:(qb4 + 1) * 128],
                        rhs=wo[:, h, ec * QCH:(ec + 1) * QCH],
                        start=(h == 0), stop=(h == HLOC - 1))
                oc = oc_pool.tile([128, QCH], BF16, tag="oc", name="oc")
                if FLAGS["evac_act"]:
                    nc.scalar.copy(oc, op_ps)
                else:
                    nc.vector.tensor_copy(oc, op_ps)
                if FLAGS["io_dma"]:
                    eng = nc.gpsimd if FLAGS["out_gpsimd"] else nc.sync
                    eng.dma_start(out=outp[qb, ec], in_=oc)
            return emit

        for qb4 in range(QCH // 128):
            for ec in range(D // QCH):
                wo_q.append(make_wo_unit(qc, ocur, qb4, ec))
        if not FLAGS["wo_defer"]:
            while norm_q:
                emit_normalize()
            while wo_q:
                wo_q.pop(0)()

    # ---- drain the tail: last chunk's normalize + Wo ----
    while norm_q:
        emit_normalize()
    while wo_q:
        wo_q.pop(0)()


def build_nc(repeat=1):
    key = (repeat, tuple(sorted(FLAGS.items())))
    if key in _BUILD_CACHE:
        return _BUILD_CACHE[key]
    nc = bacc.Bacc("TRN2", target_bir_lowering=False, debug=False,
                   num_devices=N_CORES)
    if FLAGS["timing_io"]:
        kind = "Internal"
        dummy_in = nc.dram_tensor("dummy_in", [1, 4], F32, kind="ExternalInput")
        dummy_out = nc.dram_tensor("dummy_out", [1, 4], F32, kind="ExternalOutput")
    else:
        kind = "ExternalInput"
    xT = nc.dram_tensor("xT", [NQC, NEB, 128, QCH], BF16, kind=kind)
    wqT = nc.dram_tensor("wqT", [NEB, 128, DLOC], BF16, kind=kind)
    wkT = nc.dram_tensor("wkT", [NEB, 128, DLOC], BF16, kind=kind)
    wvT = nc.dram_tensor("wvT", [NEB, 128, DLOC], BF16, kind=kind)
    woT = nc.dram_tensor("woT", [HLOC, 128, D], BF16, kind=kind)
    cosT = nc.dram_tensor("cosT", [HD, S], BF16, kind=kind)
    sinT = nc.dram_tensor("sinT", [HD, S], BF16, kind=kind)
    rT = nc.dram_tensor("rT", [HD, HD], BF16, kind=kind)
    amB = nc.dram_tensor("amB", [128, NKB], F32, kind=kind)
    eamF = nc.dram_tensor("eamF", [128, NKB], F32, kind=kind)
    eamH = nc.dram_tensor("eamH", [128, NKB], BF16, kind=kind)
    ctri = nc.dram_tensor("tri", [128, 4 * QCH], BF16, kind=kind)
    ident = nc.dram_tensor("ident", [128, 128], BF16, kind=kind)
    if FLAGS["timing_io"]:
        outp = nc.dram_tensor("outp", [S // 128, D // QCH, 128, QCH], BF16,
                              kind="Internal")
    else:
        outp = nc.dram_tensor("outp", [S // 128, D // QCH, 128, QCH], BF16,
                              kind="ExternalOutput")
    tensors = (xT, wqT, wkT, wvT, woT, cosT, sinT, rT, amB, eamF, eamH,
               ctri, ident, outp)

    from contextlib import ExitStack
    with tile.TileContext(nc) as tc, ExitStack() as ctx:
        consts = ctx.enter_context(tc.tile_pool(name="consts", bufs=1))
        resid = ctx.enter_context(tc.tile_pool(name="resid", bufs=1))
        xc_pool = ctx.enter_context(tc.tile_pool(name="xc", bufs=20))
        ps_pool = ctx.enter_context(tc.tile_pool(name="ps", bufs=4, space="PSUM"))
        work = ctx.enter_context(tc.tile_pool(name="work", bufs=2))
        p_pool = ctx.enter_context(tc.tile_pool(name="p", bufs=6))
        rb_pool = ctx.enter_context(tc.tile_pool(name="rb", bufs=2))
        oc_pool = ctx.enter_context(tc.tile_pool(name="oc", bufs=3))
        qcur_pool = ctx.enter_context(tc.tile_pool(name="qcur", bufs=8))
        ocur_pool = ctx.enter_context(tc.tile_pool(name="ocur", bufs=8))
        pacc_pool = ctx.enter_context(tc.tile_pool(name="pacc", bufs=3))
        pools = (consts, resid, xc_pool, ps_pool, work, p_pool, rb_pool,
                 oc_pool, qcur_pool, ocur_pool, pacc_pool)
        _emit_consts(nc, tc, pools, tensors)
        if FLAGS["timing_io"]:
            dsb = pools[4].tile([1, 4], F32, tag="dummy", name="dummy")
            nc.sync.dma_start(out=dsb, in_=dummy_in[:])
            nc.sync.dma_start(out=dummy_out[:], in_=dsb)
        if repeat == 1:
            _emit_body(nc, tc, pools, tensors)
        else:
            with tc.For_i(0, repeat, 1, hint_engines=(mybir.EngineType.PE, mybir.EngineType.DVE, mybir.EngineType.Activation)):
                _emit_body(nc, tc, pools, tensors)
    nc.compile()
    _BUILD_CACHE[key] = nc
    return nc


def make_core_inputs(hidden_states, attention_mask, Wq, Wk, Wv, Wo):
    """Host-side prep: returns list of 8 in_maps."""
    f32 = np.float32
    bf16 = ml_dtypes.bfloat16
    hidden_states = np.asarray(hidden_states, dtype=f32)
    attention_mask = np.asarray(attention_mask, dtype=f32)
    Wq = np.asarray(Wq, dtype=f32)
    Wk = np.asarray(Wk, dtype=f32)
    Wv = np.asarray(Wv, dtype=f32)
    Wo = np.asarray(Wo, dtype=f32)

    # rope tables, [hd, S] layout
    invf = 1.0 / (ROPE_THETA ** (np.arange(0, HD, 2, dtype=f32) / HD))
    t = np.arange(S, dtype=f32)
    fr = t[:, None] * invf[None, :]            # [S, hd/2]
    emb = np.concatenate([fr, fr], axis=-1)    # [S, hd]
    cosT = np.cos(emb).T.astype(bf16).copy()   # [hd, S]
    sinT = np.sin(emb).T.astype(bf16).copy()

    # rotate-half matrix: (R @ x)[i] = -x[i+64] (i<64), x[i-64] (i>=64)
    R = np.zeros((HD, HD), dtype=f32)
    half = HD // 2
    for i in range(half):
        R[i, i + half] = -1.0
        R[i + half, i] = 1.0
    rT = R.T.copy()

    # causal additive triangle for the diagonal 128x128 sub-block
    p = np.arange(128)[:, None]
    c = np.arange(QCH)[None, :]
    tri = np.zeros((128, 4, QCH), dtype=np.float32)
    for j in range(4):
        qrel = c - 128 * j
        tri[:, j, :] = np.where((qrel >= 0) & (qrel < 128) & (p > qrel), NEG, 0.0)
    tri = tri.reshape(128, 4 * QCH).astype(bf16)
    ident = np.eye(128, dtype=np.float32).astype(bf16)

    scale = 1.0 / math.sqrt(HD)
    in_maps = []
    for core in range(N_CORES):
        b = core // (N_CORES // B)
        hg = core % (N_CORES // B)
        rows = slice(hg * DLOC, (hg + 1) * DLOC)
        amv = np.where(attention_mask[b] == 0, NEG, attention_mask[b]).astype(f32)
        eam_bf = np.exp(amv).astype(bf16)          # e^am, bf16-rounded
        eam = eam_bf.astype(f32)                   # identical values in f32
        in_maps.append({
            "xT": np.ascontiguousarray(
                hidden_states[b].T.reshape(NEB, 128, NQC, QCH)
                .transpose(2, 0, 1, 3)).astype(bf16),
            "wqT": (Wq[rows, :] * scale).T.reshape(NEB, 128, DLOC).astype(bf16),
            "wkT": Wk[rows, :].T.reshape(NEB, 128, DLOC).astype(bf16),
            "wvT": Wv[rows, :].T.reshape(NEB, 128, DLOC).astype(bf16),
            "woT": Wo[:, rows].T.reshape(HLOC, 128, D).astype(bf16),
            "cosT": cosT,
            "sinT": sinT,
            "rT": rT.astype(bf16),
            "amB": amv.reshape(NKB, 128).T.copy(),
            "eamF": eam.reshape(NKB, 128).T.copy(),
            "eamH": eam_bf.reshape(NKB, 128).T.copy(),
            "tri": tri,
            "ident": ident,
        })
    return in_maps


def kernel(**inputs):
    nc = build_nc()
    in_maps = make_core_inputs(**inputs)
    res = run_bass_kernel_spmd(nc, in_maps, list(range(N_CORES)))
    out = np.zeros((B, S, D), dtype=np.float32)
    ncb = N_CORES // B
    for core in range(N_CORES):
        r = res.results[core]["outp"]          # [16, 4, 128, 512] tiled, bf16
        out[core // ncb] += r.astype(np.float32).transpose(0, 2, 1, 3).reshape(S, D)
    return out

